# revision 4
# baseline (speedup 1.0000x reference)
"""MoE grouped-GEMM (SwiGLU MLP, 16 experts) for 8 Trainium2 NeuronCores.

Expert-parallel as the baseline (core c owns experts {2c, 2c+1}), but all
three matmuls run in fp8e4 DoubleRow mode (2 contraction k-tiles per
instruction at 0.5 cycles/output-row). To stay within the 2e-2 error gate,
every tensor T is carried as a same-scale residual pair
    T_hi = e4m3(T*s),  T_lo = e4m3(T*s - T_hi)
and each logical matmul x@w is computed as the 3-term sum
    x_hi@w_hi + (x_hi@w_lo + x_lo@w_hi)
(the lo*lo term, ~0.07%^2, is dropped). The pair terms share scale s, so all
terms accumulate into one PSUM group. Layout packs hi/lo adjacent so a
single SBUF tile serves both the "main" (hi,hi over an ho-pair) and the
"correction" (lo,hi within one ho) DoubleRow access patterns:

  x SBUF tile  [P, HO, 2, T]   c=0:hi 1:lo
  w SBUF tile  [P, KO, 2, P]   c=0:lo 1:hi   (k-tiles KO=16 phase1, 12 phase2)
  main AP  over (2h,c=hi),(2h+1,c=hi)  - dim1 stride 2*inner
  corr AP  over (ho,c=0),(ho,c=1)      - dim1 stride inner
    pairing: ko0 = x_hi*w_lo, ko1 = x_lo*w_hi

h = silu(gate)*up is produced on-chip in fp32, scaled to SH units, and
split hi/lo by Scalar(Copy->e4m3) + DVE(subtract->e4m3).
"""

import numpy as np
import ml_dtypes

E4 = ml_dtypes.float8_e4m3  # TRN FP8_EXP4 (max +-240)
FMAX = 240.0

NUM_EXPERTS = 16
HIDDEN = 2048
INTER = 1408
TOKENS = 32768
N_CORES = 8
E_PER = NUM_EXPERTS // N_CORES  # 2
GROUP = TOKENS // NUM_EXPERTS   # 2048

P = 128
HO = HIDDEN // P       # 16 k-tiles, phase 1
HP = HO // 2           # 8 ho-pairs
IO = INTER // P        # 11
IOP = 12               # padded to even
IP = IOP // 2          # 6 i-pairs, phase 2
TN = 512
TB = GROUP // TN       # 4

# quantization scales (distribution-based, ~5-sigma to ~200 of 240 range)
SX = 40.0        # x ~ N(0,1)
SW = 4000.0      # w ~ N(0, 0.01)
SH = 24.0        # h = silu(g)*up, global max|h| ~ 4.3 -> max ~103 of 240
# folded constants
ALPHA = 1.0 / (SX * SW)           # sigmoid input unscale
CPU_S = SH / (SX * SX * SW * SW)  # pu -> pu_scaled so sil*pu_s = h*SH
COUT = 1.0 / (SH * SW)            # psum -> true out

_prog_cache = {}


def _build_program():
    import concourse.bacc as bacc
    import concourse.mybir as mybir
    import concourse.tile as tile

    f32 = mybir.dt.float32
    fp8 = mybir.dt.float8e4
    DR = mybir.MatmulPerfMode.DoubleRow

    nc = bacc.Bacc("TRN2", target_bir_lowering=False, debug=False)

    # DRAM layouts chosen so each SBUF tile loads with large contiguous rows
    xt_d = nc.dram_tensor("xt", [E_PER, P, HO, 2, GROUP], fp8, kind="ExternalInput")
    wg_d = nc.dram_tensor("wg", [E_PER, IO, P, HO, 2, P], fp8, kind="ExternalInput")
    wu_d = nc.dram_tensor("wu", [E_PER, IO, P, HO, 2, P], fp8, kind="ExternalInput")
    wd_d = nc.dram_tensor("wd", [E_PER, HO, P, IOP, 2, P], fp8, kind="ExternalInput")
    y_d = nc.dram_tensor("y", [E_PER, HO, P, GROUP], f32, kind="ExternalOutput")

    with tile.TileContext(nc) as tc:
        with (
            tc.tile_pool(name="xt", bufs=1) as xt_pool,
            tc.tile_pool(name="ht", bufs=1) as ht_pool,
            tc.tile_pool(name="wg", bufs=2) as wg_pool,
            tc.tile_pool(name="wu", bufs=2) as wu_pool,
            tc.tile_pool(name="wd", bufs=3) as wd_pool,
            tc.tile_pool(name="sig", bufs=3) as sig_pool,
            tc.tile_pool(name="sil", bufs=3) as sil_pool,
            tc.tile_pool(name="pus", bufs=3) as pus_pool,
            tc.tile_pool(name="htf", bufs=3) as htf_pool,
            tc.tile_pool(name="out", bufs=6) as out_pool,
            tc.tile_pool(name="pg", bufs=2, space="PSUM") as pg_pool,
            tc.tile_pool(name="pu", bufs=2, space="PSUM") as pu_pool,
            tc.tile_pool(name="po", bufs=4, space="PSUM") as po_pool,
        ):
            for e in range(E_PER):
                # prefetch first phase-1 weights, with x's first hi chunk
                # interleaved so the gate mains can start earliest
                wgt0 = wg_pool.tile([P, HO, 2, P], fp8, tag="wg")
                xt = xt_pool.tile([P, HO, 2, GROUP], fp8, tag="xt", name=f"xt_{e}")
                nc.sync.dma_start(wgt0[:, 0:8], wg_d[e, 0, :, 0:8])
                nc.sync.dma_start(xt[:, 0:8, 0, 0:TN], xt_d[e, :, 0:8, 0, 0:TN])
                nc.sync.dma_start(wgt0[:, 8:], wg_d[e, 0, :, 8:])
                nc.sync.dma_start(xt[:, 8:, 0, 0:TN], xt_d[e, :, 8:, 0, 0:TN])
                wut0 = wu_pool.tile([P, HO, 2, P], fp8, tag="wu")
                nc.sync.dma_start(wut0[:], wu_d[e, 0])

                # rest of x (hi+lo interleaved), tb-major so compute starts
                # after ~1/4 of the 8.4MB
                for tb in range(TB):
                    ts = slice(tb * TN, (tb + 1) * TN)
                    for c in range(2):  # hi first: main terms unblock sooner
                        if tb == 0 and c == 0:
                            continue
                        if e == 0:
                            # quarter-granular lo / half-granular hi chunks so
                            # io0's matmuls track the stream with minimal lag
                            nh = 4 if c == 1 else 2
                            for q in range(nh):
                                hs = slice(q * (HO // nh), (q + 1) * (HO // nh))
                                nc.sync.dma_start(
                                    xt[:, hs, c, ts], xt_d[e, :, hs, c, ts]
                                )
                        else:
                            nc.sync.dma_start(
                                xt[:, :, c, ts], xt_d[e, :, :, c, ts]
                            )

                # h pair tile; slot [11, hi] gets a copy of h_lo[10] and the
                # host puts wd_lo[10] in wd's [11, hi] slot, so the main pair
                # (10,11) contributes h_hi[10]*wd_hi[10] + h_lo[10]*wd_lo[10]
                ht = ht_pool.tile([P, IOP, 2, GROUP], fp8, tag="ht", name=f"ht_{e}")

                # ---- phase 1: h = silu(g)*up, g/up via 3-term fp8 DR ----
                for io in range(IO):
                    if io == 0:
                        wgt, wut = wgt0, wut0
                    else:
                        wgt = wg_pool.tile([P, HO, 2, P], fp8, tag="wg")
                        nc.sync.dma_start(wgt[:], wg_d[e, io])
                        wut = wu_pool.tile([P, HO, 2, P], fp8, tag="wu")
                        nc.sync.dma_start(wut[:], wu_d[e, io])
                    for tb in range(TB):
                        ts = slice(tb * TN, (tb + 1) * TN)
                        pg = pg_pool.tile([P, TN], f32, tag="pg")
                        pu = pu_pool.tile([P, TN], f32, tag="pu")
                        # mains first (need only x_hi), then corrections
                        for ps, wt in ((pg, wgt), (pu, wut)):
                            for hp in range(HP):
                                nc.tensor.matmul(
                                    ps[:],
                                    wt[:, 2 * hp : 2 * hp + 2, 1],
                                    xt[:, 2 * hp : 2 * hp + 2, 0, ts],
                                    start=(hp == 0), stop=False, perf_mode=DR,
                                )
                        # corr: ko0 = x_hi*w_lo, ko1 = x_lo*w_hi
                        # (gate/up interleaved per ho so correction work can
                        # track the x_lo stream during the first io of e0)
                        for ho in range(HO):
                            for ps, wt in ((pg, wgt), (pu, wut)):
                                nc.tensor.matmul(
                                    ps[:],
                                    wt[:, ho, :],
                                    xt[:, ho, :, ts],
                                    start=False, stop=(ho == HO - 1),
                                    perf_mode=DR,
                                )
                        sig = sig_pool.tile([P, TN], f32, tag="sig")
                        nc.scalar.activation(
                            sig[:], pg[:], mybir.ActivationFunctionType.Sigmoid,
                            scale=ALPHA,
                        )
                        pus = pus_pool.tile([P, TN], f32, tag="pus")
                        nc.scalar.activation(
                            pus[:], pu[:], mybir.ActivationFunctionType.Copy,
                            scale=CPU_S,
                        )
                        sil = sil_pool.tile([P, TN], f32, tag="sil")
                        nc.vector.tensor_tensor(
                            sil[:], sig[:], pg[:], mybir.AluOpType.mult
                        )
                        htf = htf_pool.tile([P, TN], f32, tag="htf")
                        nc.vector.tensor_tensor(
                            htf[:], sil[:], pus[:], mybir.AluOpType.mult
                        )
                        # split h into e4m3 pair: hi via Scalar, lo via DVE
                        nc.scalar.activation(
                            ht[:, io, 0, ts], htf[:],
                            mybir.ActivationFunctionType.Copy,
                        )
                        nc.vector.tensor_tensor(
                            ht[:, io, 1, ts], htf[:], ht[:, io, 0, ts],
                            mybir.AluOpType.subtract,
                        )
                        if io == IO - 1:
                            # duplicate h_lo[10] into the pad slot [11, hi]
                            nc.vector.tensor_tensor(
                                ht[:, IO, 0, ts], htf[:], ht[:, io, 0, ts],
                                mybir.AluOpType.subtract,
                            )

                # ---- phase 2: out = h @ wd, 3-term fp8 DR ----
                for jo in range(HO):
                    wdt = wd_pool.tile([P, IOP, 2, P], fp8, tag="wd")
                    nc.sync.dma_start(wdt[:], wd_d[e, jo])
                    last_jo = e == E_PER - 1 and jo == HO - 1
                    chunks = [slice(t * TN, (t + 1) * TN) for t in range(TB)]
                    if last_jo:
                        # split the final store so the end-of-kernel
                        # drain->act->dma tail chain is shorter
                        ts3 = chunks.pop()
                        chunks += [slice(ts3.start, ts3.start + 256),
                                   slice(ts3.start + 256, ts3.stop)]
                    for ts in chunks:
                        tn = ts.stop - ts.start
                        po = po_pool.tile([P, tn], f32, tag="po")
                        n = 0
                        for ip in range(IP):
                            nc.tensor.matmul(
                                po[:],
                                wdt[:, 2 * ip : 2 * ip + 2, 1],
                                ht[:, 2 * ip : 2 * ip + 2, 0, ts],
                                start=(n == 0), stop=False, perf_mode=DR,
                            )
                            n += 1
                        for io in range(IO):  # skip zero-pad tile 11
                            n += 1
                            nc.tensor.matmul(
                                po[:],
                                wdt[:, io, :],
                                ht[:, io, :, ts],
                                start=False, stop=(n == IP + IO),
                                perf_mode=DR,
                            )
                        ot = out_pool.tile([P, tn], f32, tag="out")
                        nc.scalar.activation(
                            ot[:], po[:], mybir.ActivationFunctionType.Copy,
                            scale=COUT,
                        )
                        nc.sync.dma_start(y_d[e, jo, :, ts], ot[:])

    nc.compile()
    return nc


def _get_program():
    if "nc" not in _prog_cache:
        _prog_cache["nc"] = _build_program()
    return _prog_cache["nc"]


def _q8(v):
    return np.clip(v, -FMAX, FMAX).astype(E4)


def _pair(v, s):
    """-> hi, lo with (hi+lo)/s ~= v, both e4m3 at the same scale."""
    hi = _q8(v * s)
    lo = _q8(v * s - hi.astype(np.float32))
    return hi, lo


def _pack_inputs(hidden_states, w_gate, w_up, w_down):
    # x [T,H] -> [E, P(hp), HO, 2, GROUP]
    xh, xl = _pair(hidden_states, SX)
    x_pair = np.stack([xh, xl], axis=0)  # [2, T, H]
    xt = (
        x_pair.reshape(2, NUM_EXPERTS, GROUP, HO, P)
        .transpose(1, 4, 3, 0, 2)  # [E, P, HO, 2, GROUP]
    )
    # wg/wu [E,H,I] -> [E, IO, P(k), HO, 2(c=lo,hi), P(m)]
    def pack_w1(w):
        hi, lo = _pair(w, SW)
        wp = np.stack([lo, hi], axis=0)  # [2, E, H, I]
        return np.ascontiguousarray(
            wp.reshape(2, NUM_EXPERTS, HO, P, IO, P)
            .transpose(1, 4, 3, 2, 0, 5)  # [E, IO, P, HO, 2, P]
        )

    wg = pack_w1(w_gate)
    wu = pack_w1(w_up)
    # wd [E,I,H] -> [E, JO, P(k), IOP, 2(c=lo,hi), P(m)], i padded 1408->1536
    dh, dl = _pair(w_down, SW)
    wdp = np.stack([dl, dh], axis=0)  # [2, E, I, H]
    wdp_pad = np.zeros((2, NUM_EXPERTS, IOP * P, HIDDEN), E4)
    wdp_pad[:, :, :INTER] = wdp
    # pad i-tile 11's hi slot carries wd_lo of i-tile 10 (pairs with the
    # kernel writing h_lo[10] into ht's [11, hi] slot)
    wdp_pad[1, :, INTER:] = wdp[0, :, INTER - P :]
    wd = np.ascontiguousarray(
        wdp_pad.reshape(2, NUM_EXPERTS, IOP, P, HO, P)
        .transpose(1, 4, 3, 2, 0, 5)  # [E, JO, P, IOP, 2, P]
    )

    in_maps = []
    for c in range(N_CORES):
        es = slice(c * E_PER, (c + 1) * E_PER)
        in_maps.append(
            {
                "xt": np.ascontiguousarray(xt[es]),
                "wg": wg[es],
                "wu": wu[es],
                "wd": wd[es],
            }
        )
    return in_maps


def _unpack_output(ys):
    y = np.stack(ys).reshape(NUM_EXPERTS, HO, P, GROUP)
    return np.ascontiguousarray(
        y.transpose(0, 3, 1, 2).reshape(TOKENS, HIDDEN)
    ).astype(np.float32)


def _numpy_fallback(hidden_states, w_gate, w_up, w_down, group_sizes):
    out = np.zeros((hidden_states.shape[0], HIDDEN), np.float32)
    off = 0
    for e in range(NUM_EXPERTS):
        g = int(group_sizes[e])
        if g == 0:
            continue
        x = hidden_states[off : off + g]
        gate = x @ w_gate[e]
        up = x @ w_up[e]
        h = gate / (1.0 + np.exp(-gate)) * up
        out[off : off + g] = h @ w_down[e]
        off += g
    return out


def kernel(hidden_states, w_gate, w_up, w_down, group_sizes):
    hidden_states = np.asarray(hidden_states, np.float32)
    w_gate = np.asarray(w_gate, np.float32)
    w_up = np.asarray(w_up, np.float32)
    w_down = np.asarray(w_down, np.float32)
    group_sizes = np.asarray(group_sizes)

    if not (
        hidden_states.shape == (TOKENS, HIDDEN)
        and np.all(group_sizes == GROUP)
    ):
        return _numpy_fallback(hidden_states, w_gate, w_up, w_down, group_sizes)

    from concourse import bass_utils

    nc = _get_program()
    in_maps = _pack_inputs(hidden_states, w_gate, w_up, w_down)
    res = bass_utils.run_bass_kernel_spmd(nc, in_maps, core_ids=list(range(N_CORES)))
    return _unpack_output([r["y"] for r in res.results])


# revision 5
# speedup vs baseline: 1.0000x; 1.0000x over previous
"""MoE grouped-GEMM (SwiGLU MLP, 16 experts) for 8 Trainium2 NeuronCores.

Expert-parallel as the baseline (core c owns experts {2c, 2c+1}), but all
three matmuls run in fp8e4 DoubleRow mode (2 contraction k-tiles per
instruction at 0.5 cycles/output-row). To stay within the 2e-2 error gate,
every tensor T is carried as a same-scale residual pair
    T_hi = e4m3(T*s),  T_lo = e4m3(T*s - T_hi)
and each logical matmul x@w is computed as the 3-term sum
    x_hi@w_hi + (x_hi@w_lo + x_lo@w_hi)
(the lo*lo term, ~0.07%^2, is dropped). The pair terms share scale s, so all
terms accumulate into one PSUM group. Layout packs hi/lo adjacent so a
single SBUF tile serves both the "main" (hi,hi over an ho-pair) and the
"correction" (lo,hi within one ho) DoubleRow access patterns:

  x SBUF tile  [P, HO, 2, T]   c=0:hi 1:lo
  w SBUF tile  [P, KO, 2, P]   c=0:lo 1:hi   (k-tiles KO=16 phase1, 12 phase2)
  main AP  over (2h,c=hi),(2h+1,c=hi)  - dim1 stride 2*inner
  corr AP  over (ho,c=0),(ho,c=1)      - dim1 stride inner
    pairing: ko0 = x_hi*w_lo, ko1 = x_lo*w_hi

h = silu(gate)*up is produced on-chip in fp32, scaled to SH units, and
split hi/lo by Scalar(Copy->e4m3) + DVE(subtract->e4m3).
"""

import numpy as np
import ml_dtypes

E4 = ml_dtypes.float8_e4m3  # TRN FP8_EXP4 (max +-240)
FMAX = 240.0

NUM_EXPERTS = 16
HIDDEN = 2048
INTER = 1408
TOKENS = 32768
N_CORES = 8
E_PER = NUM_EXPERTS // N_CORES  # 2
GROUP = TOKENS // NUM_EXPERTS   # 2048

P = 128
HO = HIDDEN // P       # 16 k-tiles, phase 1
HP = HO // 2           # 8 ho-pairs
IO = INTER // P        # 11
IOP = 12               # padded to even
IP = IOP // 2          # 6 i-pairs, phase 2
TN = 512
TB = GROUP // TN       # 4

# quantization scales (distribution-based, ~5-sigma to ~200 of 240 range)
SX = 40.0        # x ~ N(0,1)
SW = 4000.0      # w ~ N(0, 0.01)
SH = 24.0        # h = silu(g)*up, global max|h| ~ 4.3 -> max ~103 of 240
# folded constants
ALPHA = 1.0 / (SX * SW)           # sigmoid input unscale
CPU_S = SH / (SX * SX * SW * SW)  # pu -> pu_scaled so sil*pu_s = h*SH
COUT = 1.0 / (SH * SW)            # psum -> true out

_prog_cache = {}


def _build_program():
    import concourse.bacc as bacc
    import concourse.mybir as mybir
    import concourse.tile as tile

    f32 = mybir.dt.float32
    fp8 = mybir.dt.float8e4
    DR = mybir.MatmulPerfMode.DoubleRow

    nc = bacc.Bacc("TRN2", target_bir_lowering=False, debug=False)

    # DRAM layouts chosen so each SBUF tile loads with large contiguous rows
    xt_d = nc.dram_tensor("xt", [E_PER, P, HO, 2, GROUP], fp8, kind="ExternalInput")
    wg_d = nc.dram_tensor("wg", [E_PER, IO, P, HO, 2, P], fp8, kind="ExternalInput")
    wu_d = nc.dram_tensor("wu", [E_PER, IO, P, HO, 2, P], fp8, kind="ExternalInput")
    wd_d = nc.dram_tensor("wd", [E_PER, HO, P, IOP, 2, P], fp8, kind="ExternalInput")
    y_d = nc.dram_tensor("y", [E_PER, HO, P, GROUP], f32, kind="ExternalOutput")

    with tile.TileContext(nc) as tc:
        with (
            tc.tile_pool(name="xt", bufs=1) as xt_pool,
            tc.tile_pool(name="ht", bufs=1) as ht_pool,
            tc.tile_pool(name="wg", bufs=2) as wg_pool,
            tc.tile_pool(name="wu", bufs=2) as wu_pool,
            tc.tile_pool(name="wd", bufs=3) as wd_pool,
            tc.tile_pool(name="sig", bufs=3) as sig_pool,
            tc.tile_pool(name="sil", bufs=3) as sil_pool,
            tc.tile_pool(name="pus", bufs=3) as pus_pool,
            tc.tile_pool(name="htf", bufs=3) as htf_pool,
            tc.tile_pool(name="out", bufs=6) as out_pool,
            tc.tile_pool(name="pg", bufs=2, space="PSUM") as pg_pool,
            tc.tile_pool(name="pu", bufs=2, space="PSUM") as pu_pool,
            tc.tile_pool(name="po", bufs=4, space="PSUM") as po_pool,
        ):
            for e in range(E_PER):
                # prefetch first phase-1 weights, with x's first hi chunk
                # interleaved so the gate mains can start earliest
                wgt0 = wg_pool.tile([P, HO, 2, P], fp8, tag="wg")
                xt = xt_pool.tile([P, HO, 2, GROUP], fp8, tag="xt", name=f"xt_{e}")
                nc.sync.dma_start(wgt0[:, 0:8], wg_d[e, 0, :, 0:8])
                nc.sync.dma_start(xt[:, 0:8, 0, 0:TN], xt_d[e, :, 0:8, 0, 0:TN])
                nc.sync.dma_start(wgt0[:, 8:], wg_d[e, 0, :, 8:])
                nc.sync.dma_start(xt[:, 8:, 0, 0:TN], xt_d[e, :, 8:, 0, 0:TN])
                wut0 = wu_pool.tile([P, HO, 2, P], fp8, tag="wu")
                nc.sync.dma_start(wut0[:], wu_d[e, 0])

                # rest of x (hi+lo interleaved), tb-major so compute starts
                # after ~1/4 of the 8.4MB
                for tb in range(TB):
                    ts = slice(tb * TN, (tb + 1) * TN)
                    for c in range(2):  # hi first: main terms unblock sooner
                        if tb == 0 and c == 0:
                            continue
                        if e == 0:
                            # quarter-granular lo / half-granular hi chunks so
                            # io0's matmuls track the stream with minimal lag
                            nh = 4 if c == 1 else 2
                            for q in range(nh):
                                hs = slice(q * (HO // nh), (q + 1) * (HO // nh))
                                nc.sync.dma_start(
                                    xt[:, hs, c, ts], xt_d[e, :, hs, c, ts]
                                )
                        else:
                            nc.sync.dma_start(
                                xt[:, :, c, ts], xt_d[e, :, :, c, ts]
                            )

                # h pair tile; slot [11, hi] gets a copy of h_lo[10] and the
                # host puts wd_lo[10] in wd's [11, hi] slot, so the main pair
                # (10,11) contributes h_hi[10]*wd_hi[10] + h_lo[10]*wd_lo[10]
                ht = ht_pool.tile([P, IOP, 2, GROUP], fp8, tag="ht", name=f"ht_{e}")

                # ---- phase 1: h = silu(g)*up, g/up via 3-term fp8 DR ----
                for io in range(IO):
                    if io == 0:
                        wgt, wut = wgt0, wut0
                    else:
                        wgt = wg_pool.tile([P, HO, 2, P], fp8, tag="wg")
                        nc.sync.dma_start(wgt[:], wg_d[e, io])
                        wut = wu_pool.tile([P, HO, 2, P], fp8, tag="wu")
                        nc.sync.dma_start(wut[:], wu_d[e, io])
                    for tb in range(TB):
                        ts = slice(tb * TN, (tb + 1) * TN)
                        pg = pg_pool.tile([P, TN], f32, tag="pg")
                        pu = pu_pool.tile([P, TN], f32, tag="pu")
                        # mains first (need only x_hi), then corrections
                        for ps, wt in ((pg, wgt), (pu, wut)):
                            for hp in range(HP):
                                nc.tensor.matmul(
                                    ps[:],
                                    wt[:, 2 * hp : 2 * hp + 2, 1],
                                    xt[:, 2 * hp : 2 * hp + 2, 0, ts],
                                    start=(hp == 0), stop=False, perf_mode=DR,
                                )
                        # corr: ko0 = x_hi*w_lo, ko1 = x_lo*w_hi
                        # (gate/up interleaved per ho so correction work can
                        # track the x_lo stream during the first io of e0)
                        for ho in range(HO):
                            for ps, wt in ((pg, wgt), (pu, wut)):
                                nc.tensor.matmul(
                                    ps[:],
                                    wt[:, ho, :],
                                    xt[:, ho, :, ts],
                                    start=False, stop=(ho == HO - 1),
                                    perf_mode=DR,
                                )
                        sig = sig_pool.tile([P, TN], f32, tag="sig")
                        nc.scalar.activation(
                            sig[:], pg[:], mybir.ActivationFunctionType.Sigmoid,
                            scale=ALPHA,
                        )
                        pus = pus_pool.tile([P, TN], f32, tag="pus")
                        nc.scalar.activation(
                            pus[:], pu[:], mybir.ActivationFunctionType.Copy,
                            scale=CPU_S,
                        )
                        sil = sil_pool.tile([P, TN], f32, tag="sil")
                        nc.vector.tensor_tensor(
                            sil[:], sig[:], pg[:], mybir.AluOpType.mult
                        )
                        htf = htf_pool.tile([P, TN], f32, tag="htf")
                        nc.vector.tensor_tensor(
                            htf[:], sil[:], pus[:], mybir.AluOpType.mult
                        )
                        # split h into e4m3 pair: hi via Scalar, lo via DVE
                        nc.scalar.activation(
                            ht[:, io, 0, ts], htf[:],
                            mybir.ActivationFunctionType.Copy,
                        )
                        nc.vector.tensor_tensor(
                            ht[:, io, 1, ts], htf[:], ht[:, io, 0, ts],
                            mybir.AluOpType.subtract,
                        )
                        if io == IO - 1:
                            # duplicate h_lo[10] into the pad slot [11, hi]
                            nc.vector.tensor_tensor(
                                ht[:, IO, 0, ts], htf[:], ht[:, io, 0, ts],
                                mybir.AluOpType.subtract,
                            )

                # ---- phase 2: out = h @ wd, 3-term fp8 DR ----
                for jo in range(HO):
                    wdt = wd_pool.tile([P, IOP, 2, P], fp8, tag="wd")
                    nc.sync.dma_start(wdt[:], wd_d[e, jo])
                    last_jo = e == E_PER - 1 and jo == HO - 1
                    chunks = [slice(t * TN, (t + 1) * TN) for t in range(TB)]
                    if last_jo:
                        # split the final store so the end-of-kernel
                        # drain->act->dma tail chain is shorter
                        ts3 = chunks.pop()
                        chunks += [slice(ts3.start, ts3.start + 320),
                                   slice(ts3.start + 320, ts3.stop)]
                    for ts in chunks:
                        tn = ts.stop - ts.start
                        po = po_pool.tile([P, tn], f32, tag="po")
                        n = 0
                        for ip in range(IP):
                            nc.tensor.matmul(
                                po[:],
                                wdt[:, 2 * ip : 2 * ip + 2, 1],
                                ht[:, 2 * ip : 2 * ip + 2, 0, ts],
                                start=(n == 0), stop=False, perf_mode=DR,
                            )
                            n += 1
                        for io in range(IO):  # skip zero-pad tile 11
                            n += 1
                            nc.tensor.matmul(
                                po[:],
                                wdt[:, io, :],
                                ht[:, io, :, ts],
                                start=False, stop=(n == IP + IO),
                                perf_mode=DR,
                            )
                        ot = out_pool.tile([P, tn], f32, tag="out")
                        nc.scalar.activation(
                            ot[:], po[:], mybir.ActivationFunctionType.Copy,
                            scale=COUT,
                        )
                        nc.sync.dma_start(y_d[e, jo, :, ts], ot[:])

    nc.compile()
    return nc


def _get_program():
    if "nc" not in _prog_cache:
        _prog_cache["nc"] = _build_program()
    return _prog_cache["nc"]


def _q8(v):
    return np.clip(v, -FMAX, FMAX).astype(E4)


def _pair(v, s):
    """-> hi, lo with (hi+lo)/s ~= v, both e4m3 at the same scale."""
    hi = _q8(v * s)
    lo = _q8(v * s - hi.astype(np.float32))
    return hi, lo


def _pack_inputs(hidden_states, w_gate, w_up, w_down):
    # x [T,H] -> [E, P(hp), HO, 2, GROUP]
    xh, xl = _pair(hidden_states, SX)
    x_pair = np.stack([xh, xl], axis=0)  # [2, T, H]
    xt = (
        x_pair.reshape(2, NUM_EXPERTS, GROUP, HO, P)
        .transpose(1, 4, 3, 0, 2)  # [E, P, HO, 2, GROUP]
    )
    # wg/wu [E,H,I] -> [E, IO, P(k), HO, 2(c=lo,hi), P(m)]
    def pack_w1(w):
        hi, lo = _pair(w, SW)
        wp = np.stack([lo, hi], axis=0)  # [2, E, H, I]
        return np.ascontiguousarray(
            wp.reshape(2, NUM_EXPERTS, HO, P, IO, P)
            .transpose(1, 4, 3, 2, 0, 5)  # [E, IO, P, HO, 2, P]
        )

    wg = pack_w1(w_gate)
    wu = pack_w1(w_up)
    # wd [E,I,H] -> [E, JO, P(k), IOP, 2(c=lo,hi), P(m)], i padded 1408->1536
    dh, dl = _pair(w_down, SW)
    wdp = np.stack([dl, dh], axis=0)  # [2, E, I, H]
    wdp_pad = np.zeros((2, NUM_EXPERTS, IOP * P, HIDDEN), E4)
    wdp_pad[:, :, :INTER] = wdp
    # pad i-tile 11's hi slot carries wd_lo of i-tile 10 (pairs with the
    # kernel writing h_lo[10] into ht's [11, hi] slot)
    wdp_pad[1, :, INTER:] = wdp[0, :, INTER - P :]
    wd = np.ascontiguousarray(
        wdp_pad.reshape(2, NUM_EXPERTS, IOP, P, HO, P)
        .transpose(1, 4, 3, 2, 0, 5)  # [E, JO, P, IOP, 2, P]
    )

    in_maps = []
    for c in range(N_CORES):
        es = slice(c * E_PER, (c + 1) * E_PER)
        in_maps.append(
            {
                "xt": np.ascontiguousarray(xt[es]),
                "wg": wg[es],
                "wu": wu[es],
                "wd": wd[es],
            }
        )
    return in_maps


def _unpack_output(ys):
    y = np.stack(ys).reshape(NUM_EXPERTS, HO, P, GROUP)
    return np.ascontiguousarray(
        y.transpose(0, 3, 1, 2).reshape(TOKENS, HIDDEN)
    ).astype(np.float32)


def _numpy_fallback(hidden_states, w_gate, w_up, w_down, group_sizes):
    out = np.zeros((hidden_states.shape[0], HIDDEN), np.float32)
    off = 0
    for e in range(NUM_EXPERTS):
        g = int(group_sizes[e])
        if g == 0:
            continue
        x = hidden_states[off : off + g]
        gate = x @ w_gate[e]
        up = x @ w_up[e]
        h = gate / (1.0 + np.exp(-gate)) * up
        out[off : off + g] = h @ w_down[e]
        off += g
    return out


def kernel(hidden_states, w_gate, w_up, w_down, group_sizes):
    hidden_states = np.asarray(hidden_states, np.float32)
    w_gate = np.asarray(w_gate, np.float32)
    w_up = np.asarray(w_up, np.float32)
    w_down = np.asarray(w_down, np.float32)
    group_sizes = np.asarray(group_sizes)

    if not (
        hidden_states.shape == (TOKENS, HIDDEN)
        and np.all(group_sizes == GROUP)
    ):
        return _numpy_fallback(hidden_states, w_gate, w_up, w_down, group_sizes)

    from concourse import bass_utils

    nc = _get_program()
    in_maps = _pack_inputs(hidden_states, w_gate, w_up, w_down)
    res = bass_utils.run_bass_kernel_spmd(nc, in_maps, core_ids=list(range(N_CORES)))
    return _unpack_output([r["y"] for r in res.results])
